# revision 24
# baseline (speedup 1.0000x reference)
# Trainium2 Bass kernel for nn_Encoder (dense transformer block).
#
# Sharding: data-parallel over the 8192 tokens -> 8 cores, 1024 query-tokens
# each (core c handles batch b=c//2, half h=c%2). Attention needs the full
# 2048-token context of its batch element, so each core receives the whole
# batch element, rotated so its own query tokens come first (softmax over keys
# is permutation-invariant).
#
# On-chip layout is feature-major ("everything transposed"): activations live
# as [feature_partitions, tokens]. All matmuls use weights as the stationary
# lhsT operand and activations as the streaming rhs, in bf16 with fp32 PSUM
# accumulation. Per-token reductions (LayerNorm stats, softmax sum) are
# cross-partition, computed with ones-matmuls on the PE; per-token broadcasts
# use PE-replicated stat tiles. LayerNorm gamma/beta are folded into the
# consuming weight matrices on the host; the mean-subtraction is folded into
# each projection's PSUM accumulation via a K=1 matmul with the mean row;
# 1/std is applied on the PSUM->SBUF evacuation. 1/sqrt and 1/x come from
# exp/ln on the ScalarE (shared table set).
#
# The "concat of 12 identical heads @ lin_w" collapses to
# out_head @ sum_of_12_blocks(lin_w), computed on the host.
#
# Softmax uses no max-subtraction (scores are ~N(0, 0.3), exp is safe):
# exp(scores/8) streams off the scores PSUM through the ScalarE, interleaved
# per key-tile with the attention-output accumulation so PE and ACT pipeline.
# The softmax denominator comes from an appended ones-column on the
# (transposed) V operand of the attention-output matmul.
#
# Only the query-half of x is shipped in fp32 (residual use); the full
# context ships bf16, so the blocking startup DMA is halved. The post-
# attention section (lin -> LN2 -> n2 -> fc1) runs per 512-token block so
# LayerNorm2's serial stat math hides under the other block's matmuls.

import numpy as np
import ml_dtypes

import concourse.bass as bass
import concourse.tile as tile
from concourse import mybir
from concourse import bass_utils
from concourse.masks import make_identity

F32 = mybir.dt.float32
BF16 = mybir.dt.bfloat16
AF = mybir.ActivationFunctionType
OP = mybir.AluOpType

P = 128        # partitions
D = 768        # model dim
DC = D // P    # 6 feature chunks
DQ = 64        # head dim
S = 2048       # context tokens per core
Q = 1024       # query tokens per core
H = 3072       # mlp hidden
HC = H // P    # 24 hidden chunks
KT = S // P    # 16 key tiles
NF = 512       # matmul free-dim block
SB = S // NF   # 4 context blocks
QB = Q // NF   # 2 query blocks
EPS = 1e-5
N_CORES = 8


def _emit(tc, aps, flags):
    nc = tc.nc
    has_linb = flags["has_linb"]
    has_fc2b = flags["has_fc2b"]
    # CoreSim has no Gelu; sim checks swap it for Identity on both sides
    af_gelu = AF.Identity if flags.get("gelu_identity") else AF.Gelu

    with (
        tc.tile_pool(name="wconst", bufs=1) as wconst,
        tc.tile_pool(name="outp", bufs=1) as outp,
        tc.tile_pool(name="psMM", bufs=3, space="PSUM") as psMM,
    ):
        # ---- constants / small weights ----
        ZCONST = wconst.tile([P, 1], F32, tag="zconst")
        nc.gpsimd.memset(ZCONST[:], 0.0)
        ECONST = wconst.tile([P, 1], F32, tag="econst")
        nc.gpsimd.memset(ECONST[:], EPS)
        nc.const_aps.aps[(F32, 0.0)] = ZCONST[:]
        nc.const_aps.aps[(F32, EPS)] = ECONST[:]
        ONES = wconst.tile([P, NF], BF16, tag="ones")
        nc.gpsimd.memset(ONES[:], 1.0)
        IDENT = wconst.tile([P, DQ], BF16, tag="ident")
        nc.gpsimd.memset(IDENT[:], 0.0)
        make_identity(nc, IDENT[DQ:P, :], nomemset=True)

        wkv_sb = wconst.tile([P, DC, P], BF16, tag="wkv")
        nc.gpsimd.dma_start(wkv_sb[:], aps["wkv"][:])
        wq_sb = wconst.tile([P, DC, DQ], BF16, tag="wq")
        nc.gpsimd.dma_start(wq_sb[:], aps["wq"][:])
        negkv_sb = wconst.tile([1, P], BF16, tag="negkv")
        nc.gpsimd.dma_start(negkv_sb[:], aps["negc_kv"][:])
        negq_sb = wconst.tile([1, DQ], BF16, tag="negq")
        nc.gpsimd.dma_start(negq_sb[:], aps["negc_q"][:])
        lineff_sb = wconst.tile([DQ, D], BF16, tag="lineff")
        nc.gpsimd.dma_start(lineff_sb[:], aps["lin_eff"][:])
        fc1b_sb = wconst.tile([P, HC], F32, tag="fc1b")
        nc.gpsimd.dma_start(fc1b_sb[:], aps["fc1b"][:])
        if has_linb:
            linb_sb = wconst.tile([1, D], BF16, tag="linb")
            nc.gpsimd.dma_start(linb_sb[:], aps["linb"][:])
        if has_fc2b:
            fc2b_sb = wconst.tile([1, D], BF16, tag="fc2b")
            nc.gpsimd.dma_start(fc2b_sb[:], aps["fc2b"][:])

        # OUT = attn_out + x residual, lives until the very end
        OUT = outp.tile([P, DC, Q], F32, tag="out")

        with tc.tile_pool(name="xkv", bufs=1) as xkv:
            # ---- load x: bf16 full context + f32 query half ----
            xb_sb = xkv.tile([P, DC, S], BF16, tag="xb")
            for c in range(DC):
                nc.sync.dma_start(xb_sb[:, c, :], aps["xB"][:, c, :])
            xq_sb = xkv.tile([P, DC, Q], F32, tag="xq")
            for c in range(DC):
                nc.sync.dma_start(xq_sb[:, c, :], aps["xQ"][:, c, :])
            KV = xkv.tile([P, S], BF16, tag="kv")
            Qs = xkv.tile([DQ, Q], BF16, tag="q")

            with tc.tile_pool(name="ln1tmp", bufs=1) as ln1tmp:
                MEAN = ln1tmp.tile([P, S], F32, tag="mean1")
                R1 = ln1tmp.tile([P, S], F32, tag="r1")
                # ---- LN1 stats from bf16 x (squares on the DVE) ----
                x2 = ln1tmp.tile([P, DC, S], BF16, tag="x2")
                for c in range(DC):
                    nc.vector.tensor_tensor(x2[:, c, :], xb_sb[:, c, :],
                                            xb_sb[:, c, :], OP.mult)
                MEANB = ln1tmp.tile([P, S], BF16, tag="meanb")
                for blk in range(SB):
                    bs = slice(blk * NF, (blk + 1) * NF)
                    sxp = psMM.tile([P, NF], F32, tag="mm")
                    for c in range(DC):
                        nc.tensor.matmul(sxp[:], ONES[:, 0:P],
                                         xb_sb[:, c, bs],
                                         start=(c == 0), stop=(c == DC - 1))
                    nc.scalar.activation(MEAN[:, bs], sxp[:], AF.Copy,
                                         scale=1.0 / D)
                    sx2p = psMM.tile([P, NF], F32, tag="mm")
                    for c in range(DC):
                        nc.tensor.matmul(sx2p[:], ONES[:, 0:P],
                                         x2[:, c, bs],
                                         start=(c == 0), stop=(c == DC - 1))
                    ex2 = ln1tmp.tile([P, NF], F32, tag="ex2_1")
                    nc.vector.tensor_scalar_mul(ex2[:], sx2p[:], 1.0 / D)
                    msq = ln1tmp.tile([P, NF], F32, tag="msq1")
                    nc.vector.tensor_tensor(msq[:], MEAN[:, bs], MEAN[:, bs],
                                            OP.mult)
                    nc.vector.tensor_tensor(ex2[:], ex2[:], msq[:],
                                            OP.subtract)
                    nc.scalar.activation(msq[:], ex2[:], AF.Ln, bias=EPS)
                    nc.scalar.activation(R1[:, bs], msq[:], AF.Exp,
                                         scale=-0.5)
                    nc.scalar.activation(MEANB[:, bs], MEAN[:, bs], AF.Copy)

                # ---- k,v projections (mean folded; 1/std on evacuation) ----
                for blk in range(SB):
                    bs = slice(blk * NF, (blk + 1) * NF)
                    kvp = psMM.tile([P, NF], F32, tag="mm")
                    for c in range(DC):
                        nc.tensor.matmul(kvp[:], wkv_sb[:, c, :],
                                         xb_sb[:, c, bs],
                                         start=(c == 0), stop=False)
                    nc.tensor.matmul(kvp[:], negkv_sb[:],
                                     MEANB[0:1, bs], start=False, stop=True)
                    nc.vector.tensor_tensor(KV[:, bs], kvp[:], R1[:, bs],
                                            OP.mult)
                for blk in range(QB):
                    bs = slice(blk * NF, (blk + 1) * NF)
                    qp = psMM.tile([DQ, NF], F32, tag="mm")
                    for c in range(DC):
                        nc.tensor.matmul(qp[:], wq_sb[:, c, :],
                                         xb_sb[:, c, bs],
                                         start=(c == 0), stop=False)
                    nc.tensor.matmul(qp[:], negq_sb[:],
                                     MEANB[0:1, bs], start=False, stop=True)
                    nc.vector.tensor_tensor(Qs[:, bs], qp[:], R1[0:DQ, bs],
                                            OP.mult)

            with (
                tc.tile_pool(name="attn", bufs=1) as attn,
                tc.tile_pool(name="epool", bufs=3) as epool,
            ):
                # ---- fused per-key-tile loop: scores -> exp -> v-transpose,
                #      with attention-out accumulating in held PSUM tiles ----
                VT = attn.tile([P, KT, DQ], BF16, tag="VT")
                OHu = attn.tile([DQ, Q], F32, tag="ohu")
                psS_cm = tc.tile_pool(name="psS", bufs=1, space="PSUM")
                psS = psS_cm.__enter__()
                ohp = [psMM.tile([DQ, NF], F32, tag="mm", name=f"ohp{qb}")
                       for qb in range(QB)]
                sep = [psS.tile([1, NF], F32, tag=f"se{qb}",
                                name=f"sep{qb}")
                       for qb in range(QB)]
                for kt in range(KT):
                    kts = slice(kt * P, (kt + 1) * P)
                    sp = psS.tile([P, Q], F32, tag="sp")
                    for qb in range(QB):
                        qbs = slice(qb * NF, (qb + 1) * NF)
                        nc.tensor.matmul(sp[:, qbs], KV[0:DQ, kts],
                                         Qs[:, qbs], start=True, stop=True)
                    e_t = epool.tile([P, Q], BF16, tag="e")
                    nc.scalar.activation(e_t[:], sp[:], AF.Exp, scale=0.125)
                    vtp = psMM.tile([P, DQ], BF16, tag="mm")
                    nc.tensor.transpose(vtp[:], KV[DQ:P, kts], IDENT[DQ:P, :])
                    nc.vector.tensor_copy(VT[:, kt, :], vtp[:])
                    for qb in range(QB):
                        qbs = slice(qb * NF, (qb + 1) * NF)
                        nc.tensor.matmul(ohp[qb][:], VT[:, kt, :],
                                         e_t[:, qbs],
                                         start=(kt == 0), stop=(kt == KT - 1))
                        nc.tensor.matmul(sep[qb][:], ONES[:, 0:1],
                                         e_t[:, qbs],
                                         start=(kt == 0), stop=(kt == KT - 1))
                for qb in range(QB):
                    qbs = slice(qb * NF, (qb + 1) * NF)
                    nc.vector.tensor_copy(OHu[:, qbs], ohp[qb][:])

                # 1/sumexp = exp(-ln(.)), replicated over partitions by PE
                LSE = attn.tile([1, Q], F32, tag="lse")
                for qb in range(QB):
                    qbs = slice(qb * NF, (qb + 1) * NF)
                    nc.scalar.activation(LSE[0:1, qbs], sep[qb][:], AF.Ln,
                                         bias=EPS)
                RSEr = attn.tile([1, Q], F32, tag="rser")
                nc.scalar.activation(RSEr[:], LSE[:], AF.Exp, scale=-1.0)
                RSEB = attn.tile([1, Q], BF16, tag="rseb")
                nc.vector.tensor_copy(RSEB[:], RSEr[:])
                rep = psS.tile([P, Q], F32, tag="sp", name="rep")
                for qb in range(QB):
                    qbs = slice(qb * NF, (qb + 1) * NF)
                    nc.tensor.matmul(rep[:, qbs], ONES[0:1, 0:P],
                                     RSEB[0:1, qbs], start=True, stop=True)
                OHn = attn.tile([DQ, Q], BF16, tag="ohn")
                nc.vector.tensor_tensor(OHn[:], OHu[:], rep[0:DQ, :],
                                        OP.mult)
                psS_cm.__exit__(None, None, None)

                # ---- per query-block: lin+residual -> LN2 stats -> n2,
                #      then fc1; block 1's stats hide under block 0's MMs ----
                with tc.tile_pool(name="mlp", bufs=1) as mlp:
                    MEAN2 = mlp.tile([P, Q], F32, tag="mean2")
                    R2 = mlp.tile([P, Q], F32, tag="r2")
                    N2 = mlp.tile([P, DC, Q], BF16, tag="n2")
                    G = mlp.tile([P, HC, Q], BF16, tag="g")

                    def lin_and_ln2(qb):
                        qbs = slice(qb * NF, (qb + 1) * NF)
                        # lin projection + residual
                        for mc in range(DC):
                            ms = slice(mc * P, (mc + 1) * P)
                            lp = psMM.tile([P, NF], F32, tag="mm")
                            nc.tensor.matmul(lp[:], lineff_sb[:, ms],
                                             OHn[:, qbs],
                                             start=True, stop=not has_linb)
                            if has_linb:
                                nc.tensor.matmul(lp[:], linb_sb[0:1, ms],
                                                 ONES[0:1, :],
                                                 start=False, stop=True)
                            nc.vector.tensor_tensor(OUT[:, mc, qbs], lp[:],
                                                    xq_sb[:, mc, qbs], OP.add)
                        # LN2 stats for this block (casts/evacs on ScalarE
                        # via Copy, which lives in every activation table set)
                        outb = ln2tmp.tile([P, DC, NF], BF16, tag="outb")
                        nc.scalar.activation(outb[:], OUT[:, :, qbs], AF.Copy)
                        o2 = ln2tmp.tile([P, DC, NF], BF16, tag="o2")
                        nc.vector.tensor_tensor(o2[:], outb[:], outb[:],
                                                OP.mult)
                        sxp2 = psMM.tile([P, NF], F32, tag="mm")
                        for c in range(DC):
                            nc.tensor.matmul(sxp2[:], ONES[:, 0:P],
                                             outb[:, c, :],
                                             start=(c == 0),
                                             stop=(c == DC - 1))
                        nc.scalar.activation(MEAN2[:, qbs], sxp2[:], AF.Copy,
                                             scale=1.0 / D)
                        sq2p = psMM.tile([P, NF], F32, tag="mm")
                        for c in range(DC):
                            nc.tensor.matmul(sq2p[:], ONES[:, 0:P],
                                             o2[:, c, :],
                                             start=(c == 0),
                                             stop=(c == DC - 1))
                        ex22 = ln2tmp.tile([P, NF], F32, tag="ex22")
                        nc.vector.tensor_scalar_mul(ex22[:], sq2p[:], 1.0 / D)
                        msq2 = ln2tmp.tile([P, NF], F32, tag="msq2")
                        nc.vector.tensor_tensor(msq2[:], MEAN2[:, qbs],
                                                MEAN2[:, qbs], OP.mult)
                        nc.vector.tensor_tensor(ex22[:], ex22[:], msq2[:],
                                                OP.subtract)
                        nc.scalar.activation(msq2[:], ex22[:], AF.Ln,
                                             bias=EPS)
                        nc.scalar.activation(R2[:, qbs], msq2[:], AF.Exp,
                                             scale=-0.5)
                        # n2 = outb*r2 - (mean2*r2), all-bf16 TTs
                        r2b = ln2tmp.tile([P, NF], BF16, tag="r2b")
                        nc.scalar.activation(r2b[:], R2[:, qbs], AF.Copy)
                        m2r2b = ln2tmp.tile([P, NF], BF16, tag="m2r2b")
                        nc.vector.tensor_tensor(m2r2b[:], MEAN2[:, qbs],
                                                R2[:, qbs], OP.mult)
                        nc.vector.tensor_tensor(
                            N2[:, :, qbs], outb[:],
                            r2b[:, None, :].to_broadcast((P, DC, NF)),
                            OP.mult)
                        nc.vector.tensor_tensor(
                            N2[:, :, qbs], N2[:, :, qbs],
                            m2r2b[:, None, :].to_broadcast((P, DC, NF)),
                            OP.subtract)

                    def fc1_block(qb):
                        qbs = slice(qb * NF, (qb + 1) * NF)
                        for mcp in range(HC // 2):
                            w1t = wstream1.tile([P, DC, 2, P], BF16, tag="w1")
                            nc.sync.dma_start(w1t[:], aps["fc1w"][mcp])
                            for t in range(2):
                                mc = 2 * mcp + t
                                gp = psF.tile([P, NF], F32, tag="gp")
                                for c in range(DC):
                                    nc.tensor.matmul(gp[:], w1t[:, c, t, :],
                                                     N2[:, c, qbs],
                                                     start=(c == 0),
                                                     stop=(c == DC - 1))
                                nc.scalar.activation(
                                    G[:, mc, qbs], gp[:], af_gelu,
                                    bias=bias2[:, mc:mc + 1])

                    with (
                        tc.tile_pool(name="ln2tmp", bufs=1) as ln2tmp,
                        tc.tile_pool(name="wstream1", bufs=3) as wstream1,
                        tc.tile_pool(name="psF", bufs=3,
                                     space="PSUM") as psF,
                    ):
                        r2_insts = []
                        lin_and_ln2(0)
                        lin_and_ln2(1)
                        # route the gelu bias through a copy that depends on
                        # R2 so no Gelu is scheduled before the LN2 exps --
                        # keeps the ScalarE on one activation-table set
                        junk = ln2tmp.tile([P, 1], F32, tag="junk")
                        nc.vector.tensor_scalar_mul(junk[:],
                                                    R2[:, Q - 1:Q], 0.0)
                        bias2 = ln2tmp.tile([P, HC], F32, tag="bias2")
                        nc.vector.tensor_tensor(
                            bias2[:], fc1b_sb[:],
                            junk[:].to_broadcast((P, HC)), OP.add)
                        fc1_block(0)
                        fc1_block(1)

                    # ---- fc2 + bias + residual -> store ----
                    with (
                        tc.tile_pool(name="wstream2", bufs=2) as wstream2,
                        tc.tile_pool(name="stage", bufs=2) as stage,
                    ):
                        for mc in range(DC):
                            ms = slice(mc * P, (mc + 1) * P)
                            w2t = [wstream2.tile([P, HC // 2, P], BF16,
                                                 tag="w2", name=f"w2_{half}")
                                   for half in range(2)]
                            for half in range(2):
                                nc.sync.dma_start(
                                    w2t[half][:],
                                    aps["fc2w"][mc, :, half * (HC // 2):
                                                (half + 1) * (HC // 2)])
                            for qb in range(QB):
                                qbs = slice(qb * NF, (qb + 1) * NF)
                                fp = psMM.tile([P, NF], F32, tag="mm")
                                for kc in range(HC):
                                    nc.tensor.matmul(
                                        fp[:],
                                        w2t[kc // (HC // 2)][:, kc % (HC // 2), :],
                                        G[:, kc, qbs],
                                        start=(kc == 0),
                                        stop=(kc == HC - 1 and not has_fc2b))
                                if has_fc2b:
                                    nc.tensor.matmul(fp[:], fc2b_sb[0:1, ms],
                                                     ONES[0:1, :],
                                                     start=False, stop=True)
                                st = stage.tile([P, NF], F32, tag="st")
                                nc.vector.tensor_tensor(st[:], fp[:],
                                                        OUT[:, mc, qbs],
                                                        OP.add)
                                nc.sync.dma_start(aps["outT"][:, mc, qbs],
                                                  st[:])


def _legalize_waits(raw, limit=1):
    """Split multi-wait instructions: this walrus build rejects instructions
    carrying more than one sync-wait command ("Too many sync wait commands").
    Hoist all but the last wait onto standalone EventSemaphore instructions
    inserted just before, on the same engine stream (semantically identical:
    the engine blocks on them in order)."""
    import json
    d = json.loads(raw)
    n = 0
    for fn in d["functions"]:
        for bb in fn["blocks"]:
            out = []
            for inst in bb["instructions"]:
                si = inst.get("sync_info")
                waits = (si or {}).get("on_wait") or []
                if len(waits) > limit:
                    for w in waits[:-limit]:
                        n += 1
                        out.append({
                            "debug": inst.get("debug", 0),
                            "engine": inst["engine"],
                            "ins": [],
                            "outs": [],
                            "name": f"I-waitsplit-{n}",
                            "opcode": "EventSemaphore",
                            "sync_info": {"on_update": [], "on_wait": [w]},
                        })
                    si["on_wait"] = waits[-limit:]
                out.append(inst)
            bb["instructions"] = out
    return json.dumps(d).encode()


def build_program(flags):
    nc = bass.Bass("TRN2", target_bir_lowering=False, debug=False,
                   num_devices=N_CORES)
    aps = {}
    def din(name, shape, dt):
        aps[name] = nc.dram_tensor(name, shape, dt, kind="ExternalInput").ap()
    din("xB", [P, DC, S], BF16)
    din("xQ", [P, DC, Q], F32)
    din("wkv", [P, DC, P], BF16)
    din("wq", [P, DC, DQ], BF16)
    din("negc_kv", [1, P], BF16)
    din("negc_q", [1, DQ], BF16)
    din("lin_eff", [DQ, D], BF16)
    din("fc1w", [HC // 2, P, DC, 2, P], BF16)
    din("fc1b", [P, HC], F32)
    din("fc2w", [DC, P, HC, P], BF16)
    if flags["has_linb"]:
        din("linb", [1, D], BF16)
    if flags["has_fc2b"]:
        din("fc2b", [1, D], BF16)
    aps["outT"] = nc.dram_tensor("outT", [P, DC, Q], F32,
                                 kind="ExternalOutput").ap()
    with tile.TileContext(nc) as tc:
        _emit(tc, aps, flags)
    orig_to_json = nc.to_json_bytes
    nc.to_json_bytes = lambda: _legalize_waits(orig_to_json())
    return nc


def prep_shared(inputs):
    """Host-side weight preparation (shared across cores)."""
    f32 = np.float32
    g1 = np.asarray(inputs["ln1_g"], f32)
    b1 = np.asarray(inputs["ln1_b"], f32)
    wq = np.asarray(inputs["wq"], f32)
    wk = np.asarray(inputs["wk"], f32)
    wv = np.asarray(inputs["wv"], f32)
    lin_w = np.asarray(inputs["lin_w"], f32)
    lin_b = np.asarray(inputs["lin_b"], f32)
    fc1_w = np.asarray(inputs["fc1_w"], f32)
    fc1_b = np.asarray(inputs["fc1_b"], f32)
    fc2_w = np.asarray(inputs["fc2_w"], f32)
    fc2_b = np.asarray(inputs["fc2_b"], f32)

    wkv_eff = g1[:, None] * np.concatenate([wk, wv], axis=1)      # [768, 128]
    wq_eff = g1[:, None] * wq                                     # [768, 64]
    lin_eff = lin_w.reshape(12, DQ, D).sum(0)                     # [64, 768]
    fc1_eff = g1[:, None] * fc1_w                                 # [768, 3072]
    fc1_b_eff = fc1_b + b1 @ fc1_w                                # [3072]
    # beta folded into k/v/q biases: zero when b1 == 0 (the model has no
    # q/k/v biases of its own)
    bkv = b1 @ np.concatenate([wk, wv], axis=1)
    bq = b1 @ wq
    if np.any(bkv != 0) or np.any(bq != 0):
        raise NotImplementedError(
            "nonzero ln1_b with attention projections not supported")

    bf16 = ml_dtypes.bfloat16
    shared = {
        "wkv": np.ascontiguousarray(
            wkv_eff.reshape(DC, P, P).transpose(1, 0, 2)).astype(bf16),
        "wq": np.ascontiguousarray(
            wq_eff.reshape(DC, P, DQ).transpose(1, 0, 2)).astype(bf16),
        "negc_kv": np.ascontiguousarray(
            -wkv_eff.sum(0)[None, :]).astype(bf16),
        "negc_q": np.ascontiguousarray(-wq_eff.sum(0)[None, :]).astype(bf16),
        "lin_eff": np.ascontiguousarray(lin_eff).astype(bf16),
        "fc1w": np.ascontiguousarray(
            fc1_eff.reshape(DC, P, HC // 2, 2, P).transpose(2, 1, 0, 3, 4)
        ).astype(bf16),
        "fc1b": np.ascontiguousarray(fc1_b_eff.reshape(HC, P).T),
        "fc2w": np.ascontiguousarray(
            fc2_w.reshape(HC, P, DC, P).transpose(2, 1, 0, 3)).astype(bf16),
    }
    flags = {
        "has_linb": bool(np.any(lin_b != 0)),
        "has_fc2b": bool(np.any(fc2_b != 0)),
    }
    if flags["has_linb"]:
        shared["linb"] = np.ascontiguousarray(lin_b[None, :]).astype(bf16)
    if flags["has_fc2b"]:
        shared["fc2b"] = np.ascontiguousarray(fc2_b[None, :]).astype(bf16)
    return shared, flags


def per_core_x(x, core):
    b, h = core // 2, core % 2
    xb = np.asarray(x[b], np.float32)                 # [2048, 768]
    if h:
        xb = np.concatenate([xb[Q:], xb[:Q]], axis=0)  # query tokens first
    xT = np.ascontiguousarray(
        xb.T.reshape(DC, P, S).transpose(1, 0, 2))     # [P, DC, S]
    return xT


def assemble_output(results):
    out = np.zeros((4, S, D), np.float32)
    for c in range(N_CORES):
        b, h = c // 2, c % 2
        arr = results[c]["outT"]                       # [P, DC, Q]
        blk = arr.transpose(1, 0, 2).reshape(D, Q).T   # [Q, D]
        out[b, h * Q:(h + 1) * Q] = blk
    return out


_cache = {}


def _get_program(flags):
    key = (flags["has_linb"], flags["has_fc2b"])
    if key not in _cache:
        _cache[key] = build_program(flags)
    return _cache[key]


def make_in_maps(inputs):
    shared, flags = prep_shared(inputs)
    x = np.asarray(inputs["x"], np.float32)
    in_maps = []
    for c in range(N_CORES):
        m = dict(shared)
        xT = per_core_x(x, c)
        m["xB"] = xT.astype(ml_dtypes.bfloat16)
        m["xQ"] = np.ascontiguousarray(xT[:, :, 0:Q])
        in_maps.append(m)
    return in_maps, flags


def kernel(**inputs):
    in_maps, flags = make_in_maps(inputs)
    nc = _get_program(flags)
    res = bass_utils.run_bass_kernel_spmd(
        nc, in_maps, core_ids=list(range(N_CORES)))
    return assemble_output(res.results)


if __name__ == "__main__":
    nc = build_program({"has_linb": False, "has_fc2b": False})
    print("built ok")


# revision 25
# speedup vs baseline: 1.2031x; 1.2031x over previous
# Trainium2 Bass kernel for nn_Encoder (dense transformer block).
#
# Sharding: data-parallel over the 8192 tokens -> 8 cores, 1024 query-tokens
# each (core c handles batch b=c//2, half h=c%2). Attention needs the full
# 2048-token context of its batch element, so each core receives the whole
# batch element, rotated so its own query tokens come first (softmax over keys
# is permutation-invariant).
#
# On-chip layout is feature-major ("everything transposed"): activations live
# as [feature_partitions, tokens]. All matmuls use weights as the stationary
# lhsT operand and activations as the streaming rhs, in bf16 with fp32 PSUM
# accumulation. Per-token reductions (LayerNorm stats, softmax sum) are
# cross-partition, computed with ones-matmuls on the PE; per-token broadcasts
# use PE-replicated stat tiles. LayerNorm gamma/beta are folded into the
# consuming weight matrices on the host; the mean-subtraction is folded into
# each projection's PSUM accumulation via a K=1 matmul with the mean row;
# 1/std is applied on the PSUM->SBUF evacuation. 1/sqrt and 1/x come from
# exp/ln on the ScalarE (shared table set).
#
# The "concat of 12 identical heads @ lin_w" collapses to
# out_head @ sum_of_12_blocks(lin_w), computed on the host.
#
# Softmax uses no max-subtraction (scores are ~N(0, 0.3), exp is safe):
# exp(scores/8) streams off the scores PSUM through the ScalarE, interleaved
# per key-tile with the attention-output accumulation so PE and ACT pipeline.
# The softmax denominator comes from an appended ones-column on the
# (transposed) V operand of the attention-output matmul.
#
# Only the query-half of x is shipped in fp32 (residual use); the full
# context ships bf16, so the blocking startup DMA is halved. The post-
# attention section (lin -> LN2 -> n2 -> fc1) runs per 512-token block so
# LayerNorm2's serial stat math hides under the other block's matmuls.

import numpy as np
import ml_dtypes

import concourse.bass as bass
import concourse.tile as tile
from concourse import mybir
from concourse import bass_utils
from concourse.masks import make_identity

F32 = mybir.dt.float32
BF16 = mybir.dt.bfloat16
AF = mybir.ActivationFunctionType
OP = mybir.AluOpType

P = 128        # partitions
D = 768        # model dim
DC = D // P    # 6 feature chunks
DQ = 64        # head dim
S = 2048       # context tokens per core
Q = 1024       # query tokens per core
H = 3072       # mlp hidden
HC = H // P    # 24 hidden chunks
KT = S // P    # 16 key tiles
NF = 512       # matmul free-dim block
SB = S // NF   # 4 context blocks
QB = Q // NF   # 2 query blocks
EPS = 1e-5
N_CORES = 8


def _emit(tc, aps, flags):
    nc = tc.nc
    has_linb = flags["has_linb"]
    has_fc2b = flags["has_fc2b"]
    # CoreSim has no Gelu; sim checks swap it for Identity on both sides
    af_gelu = AF.Identity if flags.get("gelu_identity") else AF.Gelu

    with (
        tc.tile_pool(name="wconst", bufs=1) as wconst,
        tc.tile_pool(name="outp", bufs=1) as outp,
        tc.tile_pool(name="psMM", bufs=3, space="PSUM") as psMM,
    ):
        # ---- constants / small weights ----
        ZCONST = wconst.tile([P, 1], F32, tag="zconst")
        nc.gpsimd.memset(ZCONST[:], 0.0)
        ECONST = wconst.tile([P, 1], F32, tag="econst")
        nc.gpsimd.memset(ECONST[:], EPS)
        nc.const_aps.aps[(F32, 0.0)] = ZCONST[:]
        nc.const_aps.aps[(F32, EPS)] = ECONST[:]
        ONES = wconst.tile([P, NF], BF16, tag="ones")
        nc.gpsimd.memset(ONES[:], 1.0)
        IDENT = wconst.tile([P, DQ], BF16, tag="ident")
        nc.gpsimd.memset(IDENT[:], 0.0)
        make_identity(nc, IDENT[DQ:P, :], nomemset=True)

        wkv_sb = wconst.tile([P, DC, P], BF16, tag="wkv")
        nc.gpsimd.dma_start(wkv_sb[:], aps["wkv"][:])
        wq_sb = wconst.tile([P, DC, DQ], BF16, tag="wq")
        nc.gpsimd.dma_start(wq_sb[:], aps["wq"][:])
        negkv_sb = wconst.tile([1, P], BF16, tag="negkv")
        nc.gpsimd.dma_start(negkv_sb[:], aps["negc_kv"][:])
        negq_sb = wconst.tile([1, DQ], BF16, tag="negq")
        nc.gpsimd.dma_start(negq_sb[:], aps["negc_q"][:])
        lineff_sb = wconst.tile([DQ, D], BF16, tag="lineff")
        nc.gpsimd.dma_start(lineff_sb[:], aps["lin_eff"][:])
        fc1b_sb = wconst.tile([P, HC], F32, tag="fc1b")
        nc.gpsimd.dma_start(fc1b_sb[:], aps["fc1b"][:])
        if has_linb:
            linb_sb = wconst.tile([1, D], BF16, tag="linb")
            nc.gpsimd.dma_start(linb_sb[:], aps["linb"][:])
        if has_fc2b:
            fc2b_sb = wconst.tile([1, D], BF16, tag="fc2b")
            nc.gpsimd.dma_start(fc2b_sb[:], aps["fc2b"][:])

        # OUT = attn_out + x residual, lives until the very end
        OUT = outp.tile([P, DC, Q], F32, tag="out")

        with tc.tile_pool(name="xkv", bufs=1) as xkv:
            # ---- load x: bf16 full context + f32 query half ----
            xb_sb = xkv.tile([P, DC, S], BF16, tag="xb")
            for c in range(DC):
                nc.sync.dma_start(xb_sb[:, c, :], aps["xB"][:, c, :])
            xq_sb = xkv.tile([P, DC, Q], F32, tag="xq")
            for c in range(DC):
                nc.sync.dma_start(xq_sb[:, c, :], aps["xQ"][:, c, :])
            KV = xkv.tile([P, S], BF16, tag="kv")
            Qs = xkv.tile([DQ, Q], BF16, tag="q")

            with tc.tile_pool(name="ln1tmp", bufs=1) as ln1tmp:
                MEAN = ln1tmp.tile([P, S], F32, tag="mean1")
                R1 = ln1tmp.tile([P, S], F32, tag="r1")
                # ---- LN1 stats from bf16 x (squares on the DVE) ----
                x2 = ln1tmp.tile([P, DC, S], BF16, tag="x2")
                for c in range(DC):
                    nc.vector.tensor_tensor(x2[:, c, :], xb_sb[:, c, :],
                                            xb_sb[:, c, :], OP.mult)
                MEANB = ln1tmp.tile([P, S], BF16, tag="meanb")
                for blk in range(SB):
                    bs = slice(blk * NF, (blk + 1) * NF)
                    sxp = psMM.tile([P, NF], F32, tag="mm")
                    for c in range(DC):
                        nc.tensor.matmul(sxp[:], ONES[:, 0:P],
                                         xb_sb[:, c, bs],
                                         start=(c == 0), stop=(c == DC - 1))
                    nc.scalar.activation(MEAN[:, bs], sxp[:], AF.Copy,
                                         scale=1.0 / D)
                    sx2p = psMM.tile([P, NF], F32, tag="mm")
                    for c in range(DC):
                        nc.tensor.matmul(sx2p[:], ONES[:, 0:P],
                                         x2[:, c, bs],
                                         start=(c == 0), stop=(c == DC - 1))
                    ex2 = ln1tmp.tile([P, NF], F32, tag="ex2_1")
                    nc.vector.tensor_scalar_mul(ex2[:], sx2p[:], 1.0 / D)
                    msq = ln1tmp.tile([P, NF], F32, tag="msq1")
                    nc.vector.tensor_tensor(msq[:], MEAN[:, bs], MEAN[:, bs],
                                            OP.mult)
                    nc.vector.tensor_tensor(ex2[:], ex2[:], msq[:],
                                            OP.subtract)
                    nc.scalar.activation(msq[:], ex2[:], AF.Ln, bias=EPS)
                    nc.scalar.activation(R1[:, bs], msq[:], AF.Exp,
                                         scale=-0.5)
                    nc.scalar.activation(MEANB[:, bs], MEAN[:, bs], AF.Copy)

                # ---- k,v projections (mean folded; 1/std on evacuation) ----
                for blk in range(SB):
                    bs = slice(blk * NF, (blk + 1) * NF)
                    kvp = psMM.tile([P, NF], F32, tag="mm")
                    for c in range(DC):
                        nc.tensor.matmul(kvp[:], wkv_sb[:, c, :],
                                         xb_sb[:, c, bs],
                                         start=(c == 0), stop=False)
                    nc.tensor.matmul(kvp[:], negkv_sb[:],
                                     MEANB[0:1, bs], start=False, stop=True)
                    nc.vector.tensor_tensor(KV[:, bs], kvp[:], R1[:, bs],
                                            OP.mult)
                for blk in range(QB):
                    bs = slice(blk * NF, (blk + 1) * NF)
                    qp = psMM.tile([DQ, NF], F32, tag="mm")
                    for c in range(DC):
                        nc.tensor.matmul(qp[:], wq_sb[:, c, :],
                                         xb_sb[:, c, bs],
                                         start=(c == 0), stop=False)
                    nc.tensor.matmul(qp[:], negq_sb[:],
                                     MEANB[0:1, bs], start=False, stop=True)
                    nc.vector.tensor_tensor(Qs[:, bs], qp[:], R1[0:DQ, bs],
                                            OP.mult)

            with (
                tc.tile_pool(name="attn", bufs=1) as attn,
                tc.tile_pool(name="epool", bufs=3) as epool,
            ):
                # ---- fused per-key-tile loop: scores -> exp -> v-transpose,
                #      with attention-out accumulating in held PSUM tiles ----
                VT = attn.tile([P, KT, DQ + 1], BF16, tag="VT")
                nc.gpsimd.memset(VT[:, :, DQ:DQ + 1], 1.0)
                OHu = attn.tile([DQ + 1, Q], F32, tag="ohu")
                psS_cm = tc.tile_pool(name="psS", bufs=2, space="PSUM")
                psS = psS_cm.__enter__()
                ohp = [psMM.tile([DQ + 1, NF], F32, tag="mm", name=f"ohp{qb}")
                       for qb in range(QB)]
                for kt in range(KT):
                    kts = slice(kt * P, (kt + 1) * P)
                    sp = psS.tile([P, Q], F32, tag="sp")
                    for qb in range(QB):
                        qbs = slice(qb * NF, (qb + 1) * NF)
                        nc.tensor.matmul(sp[:, qbs], KV[0:DQ, kts],
                                         Qs[:, qbs], start=True, stop=True)
                    e_t = epool.tile([P, Q], BF16, tag="e")
                    nc.scalar.activation(e_t[:], sp[:], AF.Exp, scale=0.125)
                    vtp = psMM.tile([P, DQ], BF16, tag="mm")
                    nc.tensor.transpose(vtp[:], KV[DQ:P, kts], IDENT[DQ:P, :])
                    nc.vector.tensor_copy(VT[:, kt, 0:DQ], vtp[:])
                    for qb in range(QB):
                        qbs = slice(qb * NF, (qb + 1) * NF)
                        nc.tensor.matmul(ohp[qb][:], VT[:, kt, :],
                                         e_t[:, qbs],
                                         start=(kt == 0), stop=(kt == KT - 1))
                for qb in range(QB):
                    qbs = slice(qb * NF, (qb + 1) * NF)
                    nc.vector.tensor_copy(OHu[:, qbs], ohp[qb][:])

                # sumexp row -> partition 0 (DMA crosses partitions),
                # 1/sumexp = exp(-ln(.)), replicated over partitions by PE
                SErow = attn.tile([1, Q], F32, tag="serow")
                nc.sync.dma_start(SErow[:], OHu[DQ:DQ + 1, :])
                LSE = attn.tile([1, Q], F32, tag="lse")
                nc.scalar.activation(LSE[:], SErow[:], AF.Ln, bias=EPS)
                RSEr = attn.tile([1, Q], F32, tag="rser")
                nc.scalar.activation(RSEr[:], LSE[:], AF.Exp, scale=-1.0)
                RSEB = attn.tile([1, Q], BF16, tag="rseb")
                nc.vector.tensor_copy(RSEB[:], RSEr[:])
                rep = psS.tile([P, Q], F32, tag="sp", name="rep")
                for qb in range(QB):
                    qbs = slice(qb * NF, (qb + 1) * NF)
                    nc.tensor.matmul(rep[:, qbs], ONES[0:1, 0:P],
                                     RSEB[0:1, qbs], start=True, stop=True)
                OHn = attn.tile([DQ, Q], BF16, tag="ohn")
                nc.vector.tensor_tensor(OHn[:], OHu[0:DQ, :], rep[0:DQ, :],
                                        OP.mult)
                psS_cm.__exit__(None, None, None)

                # ---- per query-block: lin+residual -> LN2 stats -> n2,
                #      then fc1; block 1's stats hide under block 0's MMs ----
                with tc.tile_pool(name="mlp", bufs=1) as mlp:
                    MEAN2 = mlp.tile([P, Q], F32, tag="mean2")
                    R2 = mlp.tile([P, Q], F32, tag="r2")
                    N2 = mlp.tile([P, DC, Q], BF16, tag="n2")
                    G = mlp.tile([P, HC, Q], BF16, tag="g")

                    def lin_and_ln2(qb):
                        qbs = slice(qb * NF, (qb + 1) * NF)
                        # lin projection + residual
                        for mc in range(DC):
                            ms = slice(mc * P, (mc + 1) * P)
                            lp = psMM.tile([P, NF], F32, tag="mm")
                            nc.tensor.matmul(lp[:], lineff_sb[:, ms],
                                             OHn[:, qbs],
                                             start=True, stop=not has_linb)
                            if has_linb:
                                nc.tensor.matmul(lp[:], linb_sb[0:1, ms],
                                                 ONES[0:1, :],
                                                 start=False, stop=True)
                            nc.vector.tensor_tensor(OUT[:, mc, qbs], lp[:],
                                                    xq_sb[:, mc, qbs], OP.add)
                        # LN2 stats for this block (casts/evacs on ScalarE
                        # via Copy, which lives in every activation table set)
                        outb = ln2tmp.tile([P, DC, NF], BF16, tag="outb")
                        nc.scalar.activation(outb[:], OUT[:, :, qbs], AF.Copy)
                        o2 = ln2tmp.tile([P, DC, NF], BF16, tag="o2")
                        nc.vector.tensor_tensor(o2[:], outb[:], outb[:],
                                                OP.mult)
                        sxp2 = psMM.tile([P, NF], F32, tag="mm")
                        for c in range(DC):
                            nc.tensor.matmul(sxp2[:], ONES[:, 0:P],
                                             outb[:, c, :],
                                             start=(c == 0),
                                             stop=(c == DC - 1))
                        nc.scalar.activation(MEAN2[:, qbs], sxp2[:], AF.Copy,
                                             scale=1.0 / D)
                        sq2p = psMM.tile([P, NF], F32, tag="mm")
                        for c in range(DC):
                            nc.tensor.matmul(sq2p[:], ONES[:, 0:P],
                                             o2[:, c, :],
                                             start=(c == 0),
                                             stop=(c == DC - 1))
                        ex22 = ln2tmp.tile([P, NF], F32, tag="ex22")
                        nc.vector.tensor_scalar_mul(ex22[:], sq2p[:], 1.0 / D)
                        msq2 = ln2tmp.tile([P, NF], F32, tag="msq2")
                        nc.vector.tensor_tensor(msq2[:], MEAN2[:, qbs],
                                                MEAN2[:, qbs], OP.mult)
                        nc.vector.tensor_tensor(ex22[:], ex22[:], msq2[:],
                                                OP.subtract)
                        nc.scalar.activation(msq2[:], ex22[:], AF.Ln,
                                             bias=EPS)
                        nc.scalar.activation(R2[:, qbs], msq2[:], AF.Exp,
                                             scale=-0.5)
                        # n2 = outb*r2 - (mean2*r2), all-bf16 TTs
                        r2b = ln2tmp.tile([P, NF], BF16, tag="r2b")
                        nc.scalar.activation(r2b[:], R2[:, qbs], AF.Copy)
                        m2r2b = ln2tmp.tile([P, NF], BF16, tag="m2r2b")
                        nc.vector.tensor_tensor(m2r2b[:], MEAN2[:, qbs],
                                                R2[:, qbs], OP.mult)
                        nc.vector.tensor_tensor(
                            N2[:, :, qbs], outb[:],
                            r2b[:, None, :].to_broadcast((P, DC, NF)),
                            OP.mult)
                        nc.vector.tensor_tensor(
                            N2[:, :, qbs], N2[:, :, qbs],
                            m2r2b[:, None, :].to_broadcast((P, DC, NF)),
                            OP.subtract)

                    def fc1_block(qb):
                        qbs = slice(qb * NF, (qb + 1) * NF)
                        for mcp in range(HC // 2):
                            w1t = wstream1.tile([P, DC, 2, P], BF16, tag="w1")
                            nc.sync.dma_start(w1t[:], aps["fc1w"][mcp])
                            for t in range(2):
                                mc = 2 * mcp + t
                                gp = psF.tile([P, NF], F32, tag="gp")
                                for c in range(DC):
                                    nc.tensor.matmul(gp[:], w1t[:, c, t, :],
                                                     N2[:, c, qbs],
                                                     start=(c == 0),
                                                     stop=(c == DC - 1))
                                nc.scalar.activation(
                                    G[:, mc, qbs], gp[:], af_gelu,
                                    bias=bias2[:, mc:mc + 1])

                    with (
                        tc.tile_pool(name="ln2tmp", bufs=1) as ln2tmp,
                        tc.tile_pool(name="wstream1", bufs=3) as wstream1,
                        tc.tile_pool(name="psF", bufs=3,
                                     space="PSUM") as psF,
                    ):
                        r2_insts = []
                        lin_and_ln2(0)
                        lin_and_ln2(1)
                        # route the gelu bias through a copy that depends on
                        # R2 so no Gelu is scheduled before the LN2 exps --
                        # keeps the ScalarE on one activation-table set
                        junk = ln2tmp.tile([P, 1], F32, tag="junk")
                        nc.vector.tensor_scalar_mul(junk[:],
                                                    R2[:, Q - 1:Q], 0.0)
                        bias2 = ln2tmp.tile([P, HC], F32, tag="bias2")
                        nc.vector.tensor_tensor(
                            bias2[:], fc1b_sb[:],
                            junk[:].to_broadcast((P, HC)), OP.add)
                        fc1_block(0)
                        fc1_block(1)

                    # ---- fc2 + bias + residual -> store ----
                    with (
                        tc.tile_pool(name="wstream2", bufs=2) as wstream2,
                        tc.tile_pool(name="stage", bufs=2) as stage,
                    ):
                        for mc in range(DC):
                            ms = slice(mc * P, (mc + 1) * P)
                            w2t = [wstream2.tile([P, HC // 2, P], BF16,
                                                 tag="w2", name=f"w2_{half}")
                                   for half in range(2)]
                            for half in range(2):
                                nc.sync.dma_start(
                                    w2t[half][:],
                                    aps["fc2w"][mc, :, half * (HC // 2):
                                                (half + 1) * (HC // 2)])
                            for qb in range(QB):
                                qbs = slice(qb * NF, (qb + 1) * NF)
                                fp = psMM.tile([P, NF], F32, tag="mm")
                                for kc in range(HC):
                                    nc.tensor.matmul(
                                        fp[:],
                                        w2t[kc // (HC // 2)][:, kc % (HC // 2), :],
                                        G[:, kc, qbs],
                                        start=(kc == 0),
                                        stop=(kc == HC - 1 and not has_fc2b))
                                if has_fc2b:
                                    nc.tensor.matmul(fp[:], fc2b_sb[0:1, ms],
                                                     ONES[0:1, :],
                                                     start=False, stop=True)
                                st = stage.tile([P, NF], F32, tag="st")
                                nc.vector.tensor_tensor(st[:], fp[:],
                                                        OUT[:, mc, qbs],
                                                        OP.add)
                                nc.sync.dma_start(aps["outT"][:, mc, qbs],
                                                  st[:])


def _legalize_waits(raw, limit=1):
    """Split multi-wait instructions: this walrus build rejects instructions
    carrying more than one sync-wait command ("Too many sync wait commands").
    Hoist all but the last wait onto standalone EventSemaphore instructions
    inserted just before, on the same engine stream (semantically identical:
    the engine blocks on them in order)."""
    import json
    d = json.loads(raw)
    n = 0
    for fn in d["functions"]:
        for bb in fn["blocks"]:
            out = []
            for inst in bb["instructions"]:
                si = inst.get("sync_info")
                waits = (si or {}).get("on_wait") or []
                if len(waits) > limit:
                    for w in waits[:-limit]:
                        n += 1
                        out.append({
                            "debug": inst.get("debug", 0),
                            "engine": inst["engine"],
                            "ins": [],
                            "outs": [],
                            "name": f"I-waitsplit-{n}",
                            "opcode": "EventSemaphore",
                            "sync_info": {"on_update": [], "on_wait": [w]},
                        })
                    si["on_wait"] = waits[-limit:]
                out.append(inst)
            bb["instructions"] = out
    return json.dumps(d).encode()


def build_program(flags):
    nc = bass.Bass("TRN2", target_bir_lowering=False, debug=False,
                   num_devices=N_CORES)
    aps = {}
    def din(name, shape, dt):
        aps[name] = nc.dram_tensor(name, shape, dt, kind="ExternalInput").ap()
    din("xB", [P, DC, S], BF16)
    din("xQ", [P, DC, Q], F32)
    din("wkv", [P, DC, P], BF16)
    din("wq", [P, DC, DQ], BF16)
    din("negc_kv", [1, P], BF16)
    din("negc_q", [1, DQ], BF16)
    din("lin_eff", [DQ, D], BF16)
    din("fc1w", [HC // 2, P, DC, 2, P], BF16)
    din("fc1b", [P, HC], F32)
    din("fc2w", [DC, P, HC, P], BF16)
    if flags["has_linb"]:
        din("linb", [1, D], BF16)
    if flags["has_fc2b"]:
        din("fc2b", [1, D], BF16)
    aps["outT"] = nc.dram_tensor("outT", [P, DC, Q], F32,
                                 kind="ExternalOutput").ap()
    with tile.TileContext(nc) as tc:
        _emit(tc, aps, flags)
    orig_to_json = nc.to_json_bytes
    nc.to_json_bytes = lambda: _legalize_waits(orig_to_json())
    return nc


def prep_shared(inputs):
    """Host-side weight preparation (shared across cores)."""
    f32 = np.float32
    g1 = np.asarray(inputs["ln1_g"], f32)
    b1 = np.asarray(inputs["ln1_b"], f32)
    wq = np.asarray(inputs["wq"], f32)
    wk = np.asarray(inputs["wk"], f32)
    wv = np.asarray(inputs["wv"], f32)
    lin_w = np.asarray(inputs["lin_w"], f32)
    lin_b = np.asarray(inputs["lin_b"], f32)
    fc1_w = np.asarray(inputs["fc1_w"], f32)
    fc1_b = np.asarray(inputs["fc1_b"], f32)
    fc2_w = np.asarray(inputs["fc2_w"], f32)
    fc2_b = np.asarray(inputs["fc2_b"], f32)

    wkv_eff = g1[:, None] * np.concatenate([wk, wv], axis=1)      # [768, 128]
    wq_eff = g1[:, None] * wq                                     # [768, 64]
    lin_eff = lin_w.reshape(12, DQ, D).sum(0)                     # [64, 768]
    fc1_eff = g1[:, None] * fc1_w                                 # [768, 3072]
    fc1_b_eff = fc1_b + b1 @ fc1_w                                # [3072]
    # beta folded into k/v/q biases: zero when b1 == 0 (the model has no
    # q/k/v biases of its own)
    bkv = b1 @ np.concatenate([wk, wv], axis=1)
    bq = b1 @ wq
    if np.any(bkv != 0) or np.any(bq != 0):
        raise NotImplementedError(
            "nonzero ln1_b with attention projections not supported")

    bf16 = ml_dtypes.bfloat16
    shared = {
        "wkv": np.ascontiguousarray(
            wkv_eff.reshape(DC, P, P).transpose(1, 0, 2)).astype(bf16),
        "wq": np.ascontiguousarray(
            wq_eff.reshape(DC, P, DQ).transpose(1, 0, 2)).astype(bf16),
        "negc_kv": np.ascontiguousarray(
            -wkv_eff.sum(0)[None, :]).astype(bf16),
        "negc_q": np.ascontiguousarray(-wq_eff.sum(0)[None, :]).astype(bf16),
        "lin_eff": np.ascontiguousarray(lin_eff).astype(bf16),
        "fc1w": np.ascontiguousarray(
            fc1_eff.reshape(DC, P, HC // 2, 2, P).transpose(2, 1, 0, 3, 4)
        ).astype(bf16),
        "fc1b": np.ascontiguousarray(fc1_b_eff.reshape(HC, P).T),
        "fc2w": np.ascontiguousarray(
            fc2_w.reshape(HC, P, DC, P).transpose(2, 1, 0, 3)).astype(bf16),
    }
    flags = {
        "has_linb": bool(np.any(lin_b != 0)),
        "has_fc2b": bool(np.any(fc2_b != 0)),
    }
    if flags["has_linb"]:
        shared["linb"] = np.ascontiguousarray(lin_b[None, :]).astype(bf16)
    if flags["has_fc2b"]:
        shared["fc2b"] = np.ascontiguousarray(fc2_b[None, :]).astype(bf16)
    return shared, flags


def per_core_x(x, core):
    b, h = core // 2, core % 2
    xb = np.asarray(x[b], np.float32)                 # [2048, 768]
    if h:
        xb = np.concatenate([xb[Q:], xb[:Q]], axis=0)  # query tokens first
    xT = np.ascontiguousarray(
        xb.T.reshape(DC, P, S).transpose(1, 0, 2))     # [P, DC, S]
    return xT


def assemble_output(results):
    out = np.zeros((4, S, D), np.float32)
    for c in range(N_CORES):
        b, h = c // 2, c % 2
        arr = results[c]["outT"]                       # [P, DC, Q]
        blk = arr.transpose(1, 0, 2).reshape(D, Q).T   # [Q, D]
        out[b, h * Q:(h + 1) * Q] = blk
    return out


_cache = {}


def _get_program(flags):
    key = (flags["has_linb"], flags["has_fc2b"])
    if key not in _cache:
        _cache[key] = build_program(flags)
    return _cache[key]


def make_in_maps(inputs):
    shared, flags = prep_shared(inputs)
    x = np.asarray(inputs["x"], np.float32)
    in_maps = []
    for c in range(N_CORES):
        m = dict(shared)
        xT = per_core_x(x, c)
        m["xB"] = xT.astype(ml_dtypes.bfloat16)
        m["xQ"] = np.ascontiguousarray(xT[:, :, 0:Q])
        in_maps.append(m)
    return in_maps, flags


def kernel(**inputs):
    in_maps, flags = make_in_maps(inputs)
    nc = _get_program(flags)
    res = bass_utils.run_bass_kernel_spmd(
        nc, in_maps, core_ids=list(range(N_CORES)))
    return assemble_output(res.results)


if __name__ == "__main__":
    nc = build_program({"has_linb": False, "has_fc2b": False})
    print("built ok")


# revision 27
# speedup vs baseline: 1.2783x; 1.0624x over previous
# Trainium2 Bass kernel for nn_Encoder (dense transformer block).
#
# Sharding: data-parallel over the 8192 tokens -> 8 cores, 1024 query-tokens
# each (core c handles batch b=c//2, half h=c%2). Attention needs the full
# 2048-token context of its batch element, so each core receives the whole
# batch element, rotated so its own query tokens come first (softmax over keys
# is permutation-invariant).
#
# On-chip layout is feature-major ("everything transposed"): activations live
# as [feature_partitions, tokens]. All matmuls use weights as the stationary
# lhsT operand and activations as the streaming rhs, in bf16 with fp32 PSUM
# accumulation. Per-token reductions (LayerNorm stats, softmax sum) are
# cross-partition, computed with ones-matmuls on the PE; per-token broadcasts
# use PE-replicated stat tiles. LayerNorm gamma/beta are folded into the
# consuming weight matrices on the host; the mean-subtraction is folded into
# each projection's PSUM accumulation via a K=1 matmul with the mean row;
# 1/std is applied on the PSUM->SBUF evacuation. 1/sqrt and 1/x come from
# exp/ln on the ScalarE (shared table set).
#
# The "concat of 12 identical heads @ lin_w" collapses to
# out_head @ sum_of_12_blocks(lin_w), computed on the host.
#
# Softmax uses no max-subtraction (scores are ~N(0, 0.3), exp is safe):
# exp(scores/8) streams off the scores PSUM through the ScalarE, interleaved
# per key-tile with the attention-output accumulation so PE and ACT pipeline.
# The softmax denominator comes from an appended ones-column on the
# (transposed) V operand of the attention-output matmul.
#
# Only the query-half of x is shipped in fp32 (residual use); the full
# context ships bf16, so the blocking startup DMA is halved. The post-
# attention section (lin -> LN2 -> n2 -> fc1) runs per 512-token block so
# LayerNorm2's serial stat math hides under the other block's matmuls.

import numpy as np
import ml_dtypes

import concourse.bass as bass
import concourse.tile as tile
from concourse import mybir
from concourse import bass_utils
from concourse.masks import make_identity

F32 = mybir.dt.float32
BF16 = mybir.dt.bfloat16
AF = mybir.ActivationFunctionType
OP = mybir.AluOpType

P = 128        # partitions
D = 768        # model dim
DC = D // P    # 6 feature chunks
DQ = 64        # head dim
S = 2048       # context tokens per core
Q = 1024       # query tokens per core
H = 3072       # mlp hidden
HC = H // P    # 24 hidden chunks
KT = S // P    # 16 key tiles
NF = 512       # matmul free-dim block
SB = S // NF   # 4 context blocks
QB = Q // NF   # 2 query blocks
EPS = 1e-5
N_CORES = 8


def _emit(tc, aps, flags):
    nc = tc.nc
    has_linb = flags["has_linb"]
    has_fc2b = flags["has_fc2b"]
    # CoreSim has no Gelu; sim checks swap it for Identity on both sides
    af_gelu = AF.Identity if flags.get("gelu_identity") else AF.Gelu

    with (
        tc.tile_pool(name="wconst", bufs=1) as wconst,
        tc.tile_pool(name="outp", bufs=1) as outp,
        tc.tile_pool(name="psMM", bufs=3, space="PSUM") as psMM,
    ):
        # ---- constants / small weights ----
        ZCONST = wconst.tile([P, 1], F32, tag="zconst")
        nc.gpsimd.memset(ZCONST[:], 0.0)
        ECONST = wconst.tile([P, 1], F32, tag="econst")
        nc.gpsimd.memset(ECONST[:], EPS)
        nc.const_aps.aps[(F32, 0.0)] = ZCONST[:]
        nc.const_aps.aps[(F32, EPS)] = ECONST[:]
        ONES = wconst.tile([P, NF], BF16, tag="ones")
        nc.gpsimd.memset(ONES[:], 1.0)
        ONESF = wconst.tile([1, P], F32, tag="onesf")
        nc.gpsimd.memset(ONESF[:], 1.0)
        IDENT = wconst.tile([P, DQ], BF16, tag="ident")
        nc.gpsimd.memset(IDENT[:], 0.0)
        make_identity(nc, IDENT[DQ:P, :], nomemset=True)

        wkv_sb = wconst.tile([P, DC, P], BF16, tag="wkv")
        nc.gpsimd.dma_start(wkv_sb[:], aps["wkv"][:])
        wq_sb = wconst.tile([P, DC, DQ], BF16, tag="wq")
        nc.gpsimd.dma_start(wq_sb[:], aps["wq"][:])
        negkv_sb = wconst.tile([1, P], BF16, tag="negkv")
        nc.gpsimd.dma_start(negkv_sb[:], aps["negc_kv"][:])
        negq_sb = wconst.tile([1, DQ], BF16, tag="negq")
        nc.gpsimd.dma_start(negq_sb[:], aps["negc_q"][:])
        lineff_sb = wconst.tile([DQ, D], BF16, tag="lineff")
        nc.gpsimd.dma_start(lineff_sb[:], aps["lin_eff"][:])
        fc1b_sb = wconst.tile([P, HC], F32, tag="fc1b")
        nc.gpsimd.dma_start(fc1b_sb[:], aps["fc1b"][:])
        if has_linb:
            linb_sb = wconst.tile([1, D], BF16, tag="linb")
            nc.gpsimd.dma_start(linb_sb[:], aps["linb"][:])
        if has_fc2b:
            fc2b_sb = wconst.tile([1, D], BF16, tag="fc2b")
            nc.gpsimd.dma_start(fc2b_sb[:], aps["fc2b"][:])

        # OUT = attn_out + x residual, lives until the very end
        OUT = outp.tile([P, DC, Q], F32, tag="out")

        with tc.tile_pool(name="xkv", bufs=1) as xkv:
            # ---- load x: bf16 full context + f32 query half ----
            xb_sb = xkv.tile([P, DC, S], BF16, tag="xb")
            for c in range(DC):
                for hh in range(2):
                    hs = slice(hh * (S // 2), (hh + 1) * (S // 2))
                    nc.sync.dma_start(xb_sb[:, c, hs], aps["xB"][:, c, hs])
            xq_sb = xkv.tile([P, DC, Q], F32, tag="xq")
            for c in range(DC):
                nc.sync.dma_start(xq_sb[:, c, :], aps["xQ"][:, c, :])
            KV = xkv.tile([P, S], BF16, tag="kv")
            Qs = xkv.tile([DQ, Q], BF16, tag="q")

            with tc.tile_pool(name="ln1tmp", bufs=1) as ln1tmp:
                MEAN = ln1tmp.tile([P, S], F32, tag="mean1")
                R1 = ln1tmp.tile([P, S], F32, tag="r1")
                # ---- LN1 stats from bf16 x (squares on the DVE) ----
                x2 = ln1tmp.tile([P, DC, S], BF16, tag="x2")
                for c in range(DC):
                    nc.vector.tensor_tensor(x2[:, c, :], xb_sb[:, c, :],
                                            xb_sb[:, c, :], OP.mult)
                MEANB = ln1tmp.tile([P, S], BF16, tag="meanb")
                for blk in range(SB):
                    bs = slice(blk * NF, (blk + 1) * NF)
                    sxp = psMM.tile([P, NF], F32, tag="mm")
                    for c in range(DC):
                        nc.tensor.matmul(sxp[:], ONES[:, 0:P],
                                         xb_sb[:, c, bs],
                                         start=(c == 0), stop=(c == DC - 1))
                    nc.scalar.activation(MEAN[:, bs], sxp[:], AF.Copy,
                                         scale=1.0 / D)
                    sx2p = psMM.tile([P, NF], F32, tag="mm")
                    for c in range(DC):
                        nc.tensor.matmul(sx2p[:], ONES[:, 0:P],
                                         x2[:, c, bs],
                                         start=(c == 0), stop=(c == DC - 1))
                    ex2 = ln1tmp.tile([P, NF], F32, tag="ex2_1")
                    nc.vector.tensor_scalar_mul(ex2[:], sx2p[:], 1.0 / D)
                    msq = ln1tmp.tile([P, NF], F32, tag="msq1")
                    nc.vector.tensor_tensor(msq[:], MEAN[:, bs], MEAN[:, bs],
                                            OP.mult)
                    nc.vector.tensor_tensor(ex2[:], ex2[:], msq[:],
                                            OP.subtract)
                    nc.scalar.activation(msq[:], ex2[:], AF.Ln, bias=EPS)
                    nc.scalar.activation(R1[:, bs], msq[:], AF.Exp,
                                         scale=-0.5)
                    nc.scalar.activation(MEANB[:, bs], MEAN[:, bs], AF.Copy)

                # ---- k,v projections (mean folded; 1/std on evacuation) ----
                for blk in range(SB):
                    bs = slice(blk * NF, (blk + 1) * NF)
                    kvp = psMM.tile([P, NF], F32, tag="mm")
                    for c in range(DC):
                        nc.tensor.matmul(kvp[:], wkv_sb[:, c, :],
                                         xb_sb[:, c, bs],
                                         start=(c == 0), stop=False)
                    nc.tensor.matmul(kvp[:], negkv_sb[:],
                                     MEANB[0:1, bs], start=False, stop=True)
                    nc.vector.tensor_tensor(KV[:, bs], kvp[:], R1[:, bs],
                                            OP.mult)
                for blk in range(QB):
                    bs = slice(blk * NF, (blk + 1) * NF)
                    qp = psMM.tile([DQ, NF], F32, tag="mm")
                    for c in range(DC):
                        nc.tensor.matmul(qp[:], wq_sb[:, c, :],
                                         xb_sb[:, c, bs],
                                         start=(c == 0), stop=False)
                    nc.tensor.matmul(qp[:], negq_sb[:],
                                     MEANB[0:1, bs], start=False, stop=True)
                    nc.vector.tensor_tensor(Qs[:, bs], qp[:], R1[0:DQ, bs],
                                            OP.mult)

            with (
                tc.tile_pool(name="attn", bufs=1) as attn,
                tc.tile_pool(name="epool", bufs=3) as epool,
            ):
                # ---- fused per-key-tile loop: scores -> exp -> v-transpose,
                #      with attention-out accumulating in held PSUM tiles ----
                VT = attn.tile([P, KT, DQ + 1], BF16, tag="VT")
                nc.gpsimd.memset(VT[:, :, DQ:DQ + 1], 1.0)
                OHu = attn.tile([DQ + 1, Q], F32, tag="ohu")
                psS_cm = tc.tile_pool(name="psS", bufs=2, space="PSUM")
                psS = psS_cm.__enter__()
                ohp = [psMM.tile([DQ + 1, NF], F32, tag="mm", name=f"ohp{qb}")
                       for qb in range(QB)]
                for kt in range(KT):
                    kts = slice(kt * P, (kt + 1) * P)
                    sp = psS.tile([P, Q], F32, tag="sp")
                    for qb in range(QB):
                        qbs = slice(qb * NF, (qb + 1) * NF)
                        nc.tensor.matmul(sp[:, qbs], KV[0:DQ, kts],
                                         Qs[:, qbs], start=True, stop=True)
                    e_t = epool.tile([P, Q], BF16, tag="e")
                    nc.scalar.activation(e_t[:], sp[:], AF.Exp, scale=0.125)
                    vtp = psMM.tile([P, DQ], BF16, tag="mm")
                    nc.tensor.transpose(vtp[:], KV[DQ:P, kts], IDENT[DQ:P, :])
                    nc.vector.tensor_copy(VT[:, kt, 0:DQ], vtp[:])
                    for qb in range(QB):
                        qbs = slice(qb * NF, (qb + 1) * NF)
                        nc.tensor.matmul(ohp[qb][:], VT[:, kt, :],
                                         e_t[:, qbs],
                                         start=(kt == 0), stop=(kt == KT - 1))
                # per-block: evac, sumexp row -> partition 0 (DMA crosses
                # partitions), 1/sumexp = exp(-ln(.)), PE-replicate, apply
                SErow = attn.tile([1, Q], F32, tag="serow")
                LSE = attn.tile([1, Q], F32, tag="lse")
                RSEr = attn.tile([1, Q], F32, tag="rser")
                OHn = attn.tile([DQ, Q], BF16, tag="ohn")
                rep = psS.tile([P, Q], F32, tag="sp", name="rep")
                for qb in range(QB):
                    qbs = slice(qb * NF, (qb + 1) * NF)
                    nc.vector.tensor_copy(OHu[:, qbs], ohp[qb][:])
                    nc.sync.dma_start(SErow[0:1, qbs],
                                      OHu[DQ:DQ + 1, qbs])
                    nc.scalar.activation(LSE[0:1, qbs], SErow[0:1, qbs],
                                         AF.Ln, bias=EPS)
                    nc.scalar.activation(RSEr[0:1, qbs], LSE[0:1, qbs],
                                         AF.Exp, scale=-1.0)
                    nc.tensor.matmul(rep[:, qbs], ONESF[:],
                                     RSEr[0:1, qbs], start=True, stop=True)
                    nc.vector.tensor_tensor(OHn[:, qbs], OHu[0:DQ, qbs],
                                            rep[0:DQ, qbs], OP.mult)
                psS_cm.__exit__(None, None, None)

                # ---- per query-block: lin+residual -> LN2 stats -> n2,
                #      then fc1; block 1's stats hide under block 0's MMs ----
                with tc.tile_pool(name="mlp", bufs=1) as mlp:
                    MEAN2 = mlp.tile([P, Q], F32, tag="mean2")
                    R2 = mlp.tile([P, Q], F32, tag="r2")
                    N2 = mlp.tile([P, DC, Q], BF16, tag="n2")
                    G = mlp.tile([P, HC, Q], BF16, tag="g")

                    def lin_block(qb):
                        qbs = slice(qb * NF, (qb + 1) * NF)
                        # lin projection + residual
                        for mc in range(DC):
                            ms = slice(mc * P, (mc + 1) * P)
                            lp = psMM.tile([P, NF], F32, tag="mm")
                            nc.tensor.matmul(lp[:], lineff_sb[:, ms],
                                             OHn[:, qbs],
                                             start=True, stop=not has_linb)
                            if has_linb:
                                nc.tensor.matmul(lp[:], linb_sb[0:1, ms],
                                                 ONES[0:1, :],
                                                 start=False, stop=True)
                            nc.vector.tensor_tensor(OUT[:, mc, qbs], lp[:],
                                                    xq_sb[:, mc, qbs], OP.add)

                    def ln2_block(qb):
                        qbs = slice(qb * NF, (qb + 1) * NF)
                        # LN2 stats for this block (casts/evacs on ScalarE
                        # via Copy, which lives in every activation table set)
                        outb = ln2tmp.tile([P, DC, NF], BF16, tag="outb")
                        nc.scalar.activation(outb[:], OUT[:, :, qbs], AF.Copy)
                        o2 = ln2tmp.tile([P, DC, NF], BF16, tag="o2")
                        nc.vector.tensor_tensor(o2[:], outb[:], outb[:],
                                                OP.mult)
                        sxp2 = psMM.tile([P, NF], F32, tag="mm")
                        for c in range(DC):
                            nc.tensor.matmul(sxp2[:], ONES[:, 0:P],
                                             outb[:, c, :],
                                             start=(c == 0),
                                             stop=(c == DC - 1))
                        nc.scalar.activation(MEAN2[:, qbs], sxp2[:], AF.Copy,
                                             scale=1.0 / D)
                        sq2p = psMM.tile([P, NF], F32, tag="mm")
                        for c in range(DC):
                            nc.tensor.matmul(sq2p[:], ONES[:, 0:P],
                                             o2[:, c, :],
                                             start=(c == 0),
                                             stop=(c == DC - 1))
                        ex22 = ln2tmp.tile([P, NF], F32, tag="ex22")
                        nc.vector.tensor_scalar_mul(ex22[:], sq2p[:], 1.0 / D)
                        msq2 = ln2tmp.tile([P, NF], F32, tag="msq2")
                        nc.vector.tensor_tensor(msq2[:], MEAN2[:, qbs],
                                                MEAN2[:, qbs], OP.mult)
                        nc.vector.tensor_tensor(ex22[:], ex22[:], msq2[:],
                                                OP.subtract)
                        nc.scalar.activation(msq2[:], ex22[:], AF.Ln,
                                             bias=EPS)
                        nc.scalar.activation(R2[:, qbs], msq2[:], AF.Exp,
                                             scale=-0.5)
                        # n2 = outb*r2 - (mean2*r2), all-bf16 TTs
                        r2b = ln2tmp.tile([P, NF], BF16, tag="r2b")
                        nc.scalar.activation(r2b[:], R2[:, qbs], AF.Copy)
                        m2r2b = ln2tmp.tile([P, NF], BF16, tag="m2r2b")
                        nc.vector.tensor_tensor(m2r2b[:], MEAN2[:, qbs],
                                                R2[:, qbs], OP.mult)
                        nc.vector.tensor_tensor(
                            N2[:, :, qbs], outb[:],
                            r2b[:, None, :].to_broadcast((P, DC, NF)),
                            OP.mult)
                        nc.vector.tensor_tensor(
                            N2[:, :, qbs], N2[:, :, qbs],
                            m2r2b[:, None, :].to_broadcast((P, DC, NF)),
                            OP.subtract)

                    def fc1_block(qb):
                        qbs = slice(qb * NF, (qb + 1) * NF)
                        for mcp in range(HC // 2):
                            w1t = wstream1.tile([P, DC, 2, P], BF16, tag="w1")
                            nc.sync.dma_start(w1t[:], aps["fc1w"][mcp])
                            for t in range(2):
                                mc = 2 * mcp + t
                                gp = psF.tile([P, NF], F32, tag="gp")
                                for c in range(DC):
                                    nc.tensor.matmul(gp[:], w1t[:, c, t, :],
                                                     N2[:, c, qbs],
                                                     start=(c == 0),
                                                     stop=(c == DC - 1))
                                nc.scalar.activation(
                                    G[:, mc, qbs], gp[:], af_gelu,
                                    bias=bias2[:, mc:mc + 1])

                    with (
                        tc.tile_pool(name="ln2tmp", bufs=1) as ln2tmp,
                        tc.tile_pool(name="wstream1", bufs=3) as wstream1,
                        tc.tile_pool(name="psF", bufs=3,
                                     space="PSUM") as psF,
                    ):
                        lin_block(0)
                        lin_block(1)
                        ln2_block(0)
                        ln2_block(1)
                        # route the gelu bias through a copy that depends on
                        # R2 so no Gelu is scheduled before the LN2 exps --
                        # keeps the ScalarE on one activation-table set
                        junk = ln2tmp.tile([P, 1], F32, tag="junk")
                        nc.vector.tensor_scalar_mul(junk[:],
                                                    R2[:, Q - 1:Q], 0.0)
                        bias2 = ln2tmp.tile([P, HC], F32, tag="bias2")
                        nc.vector.tensor_tensor(
                            bias2[:], fc1b_sb[:],
                            junk[:].to_broadcast((P, HC)), OP.add)
                        fc1_block(0)
                        fc1_block(1)

                    # ---- fc2 + bias + residual -> store ----
                    with (
                        tc.tile_pool(name="wstream2", bufs=8) as wstream2,
                        tc.tile_pool(name="stage", bufs=2) as stage,
                    ):
                        for mc in range(DC):
                            ms = slice(mc * P, (mc + 1) * P)
                            NQ = HC // 4
                            w2t = [wstream2.tile([P, NQ, P], BF16,
                                                 tag="w2", name=f"w2_{qtr}")
                                   for qtr in range(4)]
                            for qtr in range(4):
                                nc.sync.dma_start(
                                    w2t[qtr][:],
                                    aps["fc2w"][mc, :, qtr * NQ:
                                                (qtr + 1) * NQ])
                            for qb in range(QB):
                                qbs = slice(qb * NF, (qb + 1) * NF)
                                fp = psMM.tile([P, NF], F32, tag="mm")
                                for kc in range(HC):
                                    nc.tensor.matmul(
                                        fp[:],
                                        w2t[kc // (HC // 4)][:, kc % (HC // 4), :],
                                        G[:, kc, qbs],
                                        start=(kc == 0),
                                        stop=(kc == HC - 1 and not has_fc2b))
                                if has_fc2b:
                                    nc.tensor.matmul(fp[:], fc2b_sb[0:1, ms],
                                                     ONES[0:1, :],
                                                     start=False, stop=True)
                                st = stage.tile([P, NF], F32, tag="st")
                                nc.vector.tensor_tensor(st[:], fp[:],
                                                        OUT[:, mc, qbs],
                                                        OP.add)
                                nc.sync.dma_start(aps["outT"][:, mc, qbs],
                                                  st[:])


def _legalize_waits(raw, limit=1):
    """Split multi-wait instructions: this walrus build rejects instructions
    carrying more than one sync-wait command ("Too many sync wait commands").
    Hoist all but the last wait onto standalone EventSemaphore instructions
    inserted just before, on the same engine stream (semantically identical:
    the engine blocks on them in order)."""
    import json
    d = json.loads(raw)
    n = 0
    for fn in d["functions"]:
        for bb in fn["blocks"]:
            out = []
            for inst in bb["instructions"]:
                si = inst.get("sync_info")
                waits = (si or {}).get("on_wait") or []
                if len(waits) > limit:
                    for w in waits[:-limit]:
                        n += 1
                        out.append({
                            "debug": inst.get("debug", 0),
                            "engine": inst["engine"],
                            "ins": [],
                            "outs": [],
                            "name": f"I-waitsplit-{n}",
                            "opcode": "EventSemaphore",
                            "sync_info": {"on_update": [], "on_wait": [w]},
                        })
                    si["on_wait"] = waits[-limit:]
                out.append(inst)
            bb["instructions"] = out
    return json.dumps(d).encode()


def build_program(flags):
    nc = bass.Bass("TRN2", target_bir_lowering=False, debug=False,
                   num_devices=N_CORES)
    aps = {}
    def din(name, shape, dt):
        aps[name] = nc.dram_tensor(name, shape, dt, kind="ExternalInput").ap()
    din("xB", [P, DC, S], BF16)
    din("xQ", [P, DC, Q], F32)
    din("wkv", [P, DC, P], BF16)
    din("wq", [P, DC, DQ], BF16)
    din("negc_kv", [1, P], BF16)
    din("negc_q", [1, DQ], BF16)
    din("lin_eff", [DQ, D], BF16)
    din("fc1w", [HC // 2, P, DC, 2, P], BF16)
    din("fc1b", [P, HC], F32)
    din("fc2w", [DC, P, HC, P], BF16)
    if flags["has_linb"]:
        din("linb", [1, D], BF16)
    if flags["has_fc2b"]:
        din("fc2b", [1, D], BF16)
    aps["outT"] = nc.dram_tensor("outT", [P, DC, Q], F32,
                                 kind="ExternalOutput").ap()
    with tile.TileContext(nc) as tc:
        _emit(tc, aps, flags)
    orig_to_json = nc.to_json_bytes
    nc.to_json_bytes = lambda: _legalize_waits(orig_to_json())
    return nc


def prep_shared(inputs):
    """Host-side weight preparation (shared across cores)."""
    f32 = np.float32
    g1 = np.asarray(inputs["ln1_g"], f32)
    b1 = np.asarray(inputs["ln1_b"], f32)
    wq = np.asarray(inputs["wq"], f32)
    wk = np.asarray(inputs["wk"], f32)
    wv = np.asarray(inputs["wv"], f32)
    lin_w = np.asarray(inputs["lin_w"], f32)
    lin_b = np.asarray(inputs["lin_b"], f32)
    fc1_w = np.asarray(inputs["fc1_w"], f32)
    fc1_b = np.asarray(inputs["fc1_b"], f32)
    fc2_w = np.asarray(inputs["fc2_w"], f32)
    fc2_b = np.asarray(inputs["fc2_b"], f32)

    wkv_eff = g1[:, None] * np.concatenate([wk, wv], axis=1)      # [768, 128]
    wq_eff = g1[:, None] * wq                                     # [768, 64]
    lin_eff = lin_w.reshape(12, DQ, D).sum(0)                     # [64, 768]
    fc1_eff = g1[:, None] * fc1_w                                 # [768, 3072]
    fc1_b_eff = fc1_b + b1 @ fc1_w                                # [3072]
    # beta folded into k/v/q biases: zero when b1 == 0 (the model has no
    # q/k/v biases of its own)
    bkv = b1 @ np.concatenate([wk, wv], axis=1)
    bq = b1 @ wq
    if np.any(bkv != 0) or np.any(bq != 0):
        raise NotImplementedError(
            "nonzero ln1_b with attention projections not supported")

    bf16 = ml_dtypes.bfloat16
    shared = {
        "wkv": np.ascontiguousarray(
            wkv_eff.reshape(DC, P, P).transpose(1, 0, 2)).astype(bf16),
        "wq": np.ascontiguousarray(
            wq_eff.reshape(DC, P, DQ).transpose(1, 0, 2)).astype(bf16),
        "negc_kv": np.ascontiguousarray(
            -wkv_eff.sum(0)[None, :]).astype(bf16),
        "negc_q": np.ascontiguousarray(-wq_eff.sum(0)[None, :]).astype(bf16),
        "lin_eff": np.ascontiguousarray(lin_eff).astype(bf16),
        "fc1w": np.ascontiguousarray(
            fc1_eff.reshape(DC, P, HC // 2, 2, P).transpose(2, 1, 0, 3, 4)
        ).astype(bf16),
        "fc1b": np.ascontiguousarray(fc1_b_eff.reshape(HC, P).T),
        "fc2w": np.ascontiguousarray(
            fc2_w.reshape(HC, P, DC, P).transpose(2, 1, 0, 3)).astype(bf16),
    }
    flags = {
        "has_linb": bool(np.any(lin_b != 0)),
        "has_fc2b": bool(np.any(fc2_b != 0)),
    }
    if flags["has_linb"]:
        shared["linb"] = np.ascontiguousarray(lin_b[None, :]).astype(bf16)
    if flags["has_fc2b"]:
        shared["fc2b"] = np.ascontiguousarray(fc2_b[None, :]).astype(bf16)
    return shared, flags


def per_core_x(x, core):
    b, h = core // 2, core % 2
    xb = np.asarray(x[b], np.float32)                 # [2048, 768]
    if h:
        xb = np.concatenate([xb[Q:], xb[:Q]], axis=0)  # query tokens first
    xT = np.ascontiguousarray(
        xb.T.reshape(DC, P, S).transpose(1, 0, 2))     # [P, DC, S]
    return xT


def assemble_output(results):
    out = np.zeros((4, S, D), np.float32)
    for c in range(N_CORES):
        b, h = c // 2, c % 2
        arr = results[c]["outT"]                       # [P, DC, Q]
        blk = arr.transpose(1, 0, 2).reshape(D, Q).T   # [Q, D]
        out[b, h * Q:(h + 1) * Q] = blk
    return out


_cache = {}


def _get_program(flags):
    key = (flags["has_linb"], flags["has_fc2b"])
    if key not in _cache:
        _cache[key] = build_program(flags)
    return _cache[key]


def make_in_maps(inputs):
    shared, flags = prep_shared(inputs)
    x = np.asarray(inputs["x"], np.float32)
    in_maps = []
    for c in range(N_CORES):
        m = dict(shared)
        xT = per_core_x(x, c)
        m["xB"] = xT.astype(ml_dtypes.bfloat16)
        m["xQ"] = np.ascontiguousarray(xT[:, :, 0:Q])
        in_maps.append(m)
    return in_maps, flags


def kernel(**inputs):
    in_maps, flags = make_in_maps(inputs)
    nc = _get_program(flags)
    res = bass_utils.run_bass_kernel_spmd(
        nc, in_maps, core_ids=list(range(N_CORES)))
    return assemble_output(res.results)


if __name__ == "__main__":
    nc = build_program({"has_linb": False, "has_fc2b": False})
    print("built ok")
